# revision 7
# baseline (speedup 1.0000x reference)
"""Trainium2 Bass kernel for nn_DiTBlock_67989332295816.

Full inputs in, full outputs out. Sharding: 8 cores = (batch b = core//2) x
(sequence half s = core%2, 4096 columns each + 1 halo column on the partner
side). The MinGRU scans run as chained DVE tensor_tensor_scan chunks in
[channel-partition, L-free] layout; the cross-half scan carry is exchanged
with a tiny pair AllReduce and applied as h += cumprod(a) * carry. The k=3
depthwise convs use the halo column, so no second exchange is needed.

pixel_norm folds into host-premultiplied weights plus a per-column inorm
vector that is produced broadcast across all partitions by an all-ones
matmul. All matmuls run in bf16 (the block's modulation scales are ~0.04, so
branch signals are ~1000x smaller than the f32 residual passthrough; bf16
error is diluted to ~1e-6 of output scale).
"""
import sys

for _p in ("/opt/trn_rl_repo", "/root/.axon_site/_ro/trn_rl_repo"):
    if _p not in sys.path:
        sys.path.append(_p)

import numpy as np
import ml_dtypes
from contextlib import ExitStack

import concourse.bass as bass
import concourse.tile as tile
from concourse import bacc, mybir
from concourse.bass_utils import run_bass_kernel_spmd

F32 = mybir.dt.float32
BF16 = mybir.dt.bfloat16
NPBF = ml_dtypes.bfloat16
AL = mybir.AluOpType
AF = mybir.ActivationFunctionType

P = 128
D = 512          # model dim
KD = 4           # D / P
H = 256          # minGRU hidden per direction
MZH = 8          # ptiles of the fused [zf zf hf hf zb zb hb hb] matmul output
LLOC = 4096      # own columns per core
W = LLOC + 2     # buffer width incl. 2 halo columns (own = [1, 4097))
CH = 512
N_FULL = 9       # chunks covering [0, 4098): 8x512 + 1x2
N_OWN = 8        # chunks covering own cols [1, 4097)

TRACE = False
LAST_RESULT = None
_NC = None


def _full_chunks():
    out = []
    for c in range(N_FULL):
        off = c * CH
        out.append((off, min(CH, W - off)))
    return out


def _build():
    nc = bacc.Bacc("TRN2", target_bir_lowering=False, debug=False, num_devices=8)

    xbf_d = nc.dram_tensor("xbf", [D, W], BF16, kind="ExternalInput")
    xf_d = nc.dram_tensor("xf", [D, W], F32, kind="ExternalInput")
    wzh_d = nc.dram_tensor("wzh", [P, KD, 2 * D], BF16, kind="ExternalInput")
    bzh_d = nc.dram_tensor("bzh", [P, MZH], F32, kind="ExternalInput")
    nbzh_d = nc.dram_tensor("nbzh", [P, MZH], F32, kind="ExternalInput")
    wseq_d = nc.dram_tensor("wseq", [P, KD, D], BF16, kind="ExternalInput")
    alphas_d = nc.dram_tensor("alphas", [P, KD], F32, kind="ExternalInput")
    win_d = nc.dram_tensor("win", [P, KD, D], BF16, kind="ExternalInput")
    bin_d = nc.dram_tensor("bin", [P, KD], F32, kind="ExternalInput")
    alphac_d = nc.dram_tensor("alphac", [P, KD], F32, kind="ExternalInput")
    wpwh_d = nc.dram_tensor("wpwh", [P, KD, D], BF16, kind="ExternalInput")
    wpwg_d = nc.dram_tensor("wpwg", [P, KD, D], BF16, kind="ExternalInput")
    wout_d = nc.dram_tensor("wout", [P, KD, D], BF16, kind="ExternalInput")
    dwh_d = nc.dram_tensor("dwh", [P, KD, 3], F32, kind="ExternalInput")
    dwg_d = nc.dram_tensor("dwg", [P, KD, 3], F32, kind="ExternalInput")
    maskv_d = nc.dram_tensor("maskv", [P, 2], F32, kind="ExternalInput")
    y_d = nc.dram_tensor("y", [D, LLOC], F32, kind="ExternalOutput")

    x_view = xbf_d.ap().rearrange("(k p) l -> p k l", p=P)
    xf_view = xf_d.ap().rearrange("(k p) l -> p k l", p=P)
    y_view = y_d.ap().rearrange("(k p) l -> p k l", p=P)

    with tile.TileContext(nc) as tc, ExitStack() as ctx:
        wp = ctx.enter_context(tc.tile_pool(name="wp", bufs=1))
        res = ctx.enter_context(tc.tile_pool(name="res", bufs=1))
        ws4 = ctx.enter_context(tc.tile_pool(name="ws4", bufs=5))
        wsf = ctx.enter_context(tc.tile_pool(name="wsf", bufs=10))
        pp = ctx.enter_context(tc.tile_pool(name="pp", bufs=3, space="PSUM"))
        dram = ctx.enter_context(tc.tile_pool(name="dram", bufs=1, space="DRAM"))
        sm = ctx.enter_context(tc.tile_pool(name="sm", bufs=1))

        def psum(tag="pp"):
            return pp.tile([P, CH], F32, tag=tag, name=tag)

        def t4():  # [128, 4, 512] bf16 workspace
            return ws4.tile([P, KD, CH], BF16, tag="t4", name="t4")

        def tf():  # [128, 512] f32 workspace
            return wsf.tile([P, CH], F32, tag="tf", name="tf")

        def tb():  # [128, 512] bf16 workspace (shares ws4 pool, own tag)
            return ws4.tile([P, CH], BF16, tag="tb", name="tb")

        # ---- resident weights / consts ----
        wseq_t = wp.tile([P, KD, D], BF16, tag="wseq")
        nc.sync.dma_start(wseq_t[:], wseq_d.ap())
        win_t = wp.tile([P, KD, D], BF16, tag="win")
        nc.sync.dma_start(win_t[:], win_d.ap())
        wpwh_t = wp.tile([P, KD, D], BF16, tag="wpwh")
        nc.sync.dma_start(wpwh_t[:], wpwh_d.ap())
        wpwg_t = wp.tile([P, KD, D], BF16, tag="wpwg")
        nc.sync.dma_start(wpwg_t[:], wpwg_d.ap())
        wout_t = wp.tile([P, KD, D], BF16, tag="wout")
        nc.sync.dma_start(wout_t[:], wout_d.ap())
        bzh_t = wp.tile([P, MZH], F32, tag="bzh")
        nc.sync.dma_start(bzh_t[:], bzh_d.ap())
        nbzh_t = wp.tile([P, MZH], F32, tag="nbzh")
        nc.sync.dma_start(nbzh_t[:], nbzh_d.ap())
        alphas_t = wp.tile([P, KD], F32, tag="alphas")
        nc.sync.dma_start(alphas_t[:], alphas_d.ap())
        bin_t = wp.tile([P, KD], F32, tag="bin")
        nc.sync.dma_start(bin_t[:], bin_d.ap())
        alphac_t = wp.tile([P, KD], F32, tag="alphac")
        nc.sync.dma_start(alphac_t[:], alphac_d.ap())
        dwh_t = wp.tile([P, KD, 3], F32, tag="dwh")
        nc.sync.dma_start(dwh_t[:], dwh_d.ap())
        dwg_t = wp.tile([P, KD, 3], F32, tag="dwg")
        nc.sync.dma_start(dwg_t[:], dwg_d.ap())
        maskv_t = wp.tile([P, 2], F32, tag="maskv")
        nc.sync.dma_start(maskv_t[:], maskv_d.ap())
        ones_t = wp.tile([P, P], BF16, tag="ones")
        nc.vector.memset(ones_t[:], 1.0)
        eps_t = wp.tile([P, 1], F32, tag="eps")
        nc.vector.memset(eps_t[:], 1e-4)

        # ---- long-lived tensors ----
        h_t = [res.tile([P, W], BF16, tag=f"h{i}", name=f"h{i}") for i in range(4)]
        corr1 = res.tile([P, KD, W], BF16, tag="corr1")

        full_chunks = _full_chunks()
        own = slice(1, 1 + LLOC)

        # =================== Phase A: seq-mixer ===================
        with tc.tile_pool(name="abp", bufs=1) as abp:
            wzh_t = abp.tile([P, KD, 2 * D], BF16, tag="wzh")
            nc.sync.dma_start(wzh_t[:], wzh_d.ap())
            a_t = [abp.tile([P, W], BF16, tag=f"a{i}", name=f"a{i}") for i in range(4)]
            b_t = [abp.tile([P, W], BF16, tag=f"b{i}", name=f"b{i}") for i in range(4)]

            for off, w in full_chunks:
                xc = t4()
                nc.sync.dma_start(xc[:, :, :w], x_view[:, :, off:off + w])
                xsq = t4()
                nc.vector.tensor_mul(xsq[:, :, :w], xc[:, :, :w], xc[:, :, :w])
                ps_ss = psum()
                for k in range(KD):
                    nc.tensor.matmul(ps_ss[:, :w], ones_t[:], xsq[:, k, :w],
                                     start=(k == 0), stop=(k == KD - 1))
                inorm = tf()
                nc.scalar.activation(inorm[:, :w], ps_ss[:, :w], AF.Sqrt,
                                     bias=eps_t[:], scale=1.0 / D)
                nc.vector.reciprocal_approx_fast(inorm[:, :w], inorm[:, :w])

                z_hold = {}
                for m in range(MZH):
                    d_ = m // 4            # 0 fwd, 1 bwd
                    kind = (m % 4) // 2    # 0 -> z, 1 -> h~
                    pt = m % 2
                    i = d_ * 2 + pt
                    ps = psum()
                    for k in range(KD):
                        nc.tensor.matmul(ps[:, :w], wzh_t[:, k, m * P:(m + 1) * P],
                                         xc[:, k, :w],
                                         start=(k == 0), stop=(k == KD - 1))
                    t = tf()
                    nc.vector.tensor_mul(t[:, :w], ps[:, :w], inorm[:, :w])
                    if kind == 0:
                        nc.scalar.activation(a_t[i][:, off:off + w], t[:, :w],
                                             AF.Sigmoid, bias=nbzh_t[:, m:m + 1],
                                             scale=-1.0)
                        z = tb()
                        nc.scalar.activation(z[:, :w], t[:, :w], AF.Sigmoid,
                                             bias=bzh_t[:, m:m + 1], scale=1.0)
                        z_hold[i] = z
                    else:
                        nc.vector.scalar_tensor_tensor(
                            out=b_t[i][:, off:off + w], in0=t[:, :w],
                            scalar=bzh_t[:, m:m + 1], in1=z_hold[i][:, :w],
                            op0=AL.add, op1=AL.mult)

            # ---- scans (DVE), chained over 8 own chunks ----
            for pt in range(2):
                for c in range(N_OWN):
                    sl = slice(1 + c * CH, 1 + (c + 1) * CH)
                    init = 0.0 if c == 0 else h_t[pt][:, c * CH:c * CH + 1]
                    nc.vector.tensor_tensor_scan(
                        h_t[pt][:, sl], a_t[pt][:, sl], b_t[pt][:, sl], init,
                        AL.mult, AL.add)
                i = 2 + pt
                for c in range(N_OWN - 1, -1, -1):
                    sl = slice(1 + c * CH, 1 + (c + 1) * CH)
                    init = (0.0 if c == N_OWN - 1
                            else h_t[i][:, 1 + (c + 1) * CH:2 + (c + 1) * CH])
                    nc.vector.tensor_tensor_scan(
                        h_t[i][:, sl][:, ::-1], a_t[i][:, sl][:, ::-1],
                        b_t[i][:, sl][:, ::-1], init, AL.mult, AL.add)

            # ---- carry exchange: masked pair AllReduce of 4 columns ----
            contrib = sm.tile([P, 4], F32, tag="contrib")
            for pt in range(2):
                nc.vector.tensor_scalar_mul(contrib[:, pt:pt + 1],
                                            h_t[pt][:, LLOC:LLOC + 1],
                                            maskv_t[:, 1:2])
                nc.vector.tensor_scalar_mul(contrib[:, 2 + pt:3 + pt],
                                            h_t[2 + pt][:, 1:2],
                                            maskv_t[:, 0:1])
            cin = dram.tile([P, 4], F32, tag="cin")
            cout = dram.tile([P, 4], F32, tag="cout")
            nc.sync.dma_start(cin[:], contrib[:])
            nc.gpsimd.collective_compute(
                "AllReduce", AL.add,
                replica_groups=[[0, 1], [2, 3], [4, 5], [6, 7]],
                ins=[cin.opt()], outs=[cout.opt()])
            R = sm.tile([P, 4], F32, tag="R")
            nc.sync.dma_start(R[:], cout[:])
            cu = sm.tile([P, 4], F32, tag="cu")
            nc.vector.tensor_scalar_mul(cu[:, 0:2], R[:, 0:2], maskv_t[:, 0:1])
            nc.vector.tensor_scalar_mul(cu[:, 2:4], R[:, 2:4], maskv_t[:, 1:2])

            # ---- cumprod(a) into the dead b tiles (overlaps the collective) ----
            for pt in range(2):
                for c in range(N_OWN):
                    sl = slice(1 + c * CH, 1 + (c + 1) * CH)
                    initA = 1.0 if c == 0 else b_t[pt][:, c * CH:c * CH + 1]
                    nc.vector.tensor_tensor_scan(
                        b_t[pt][:, sl], a_t[pt][:, sl], a_t[pt][:, sl], initA,
                        AL.mult, AL.bypass)
                i = 2 + pt
                for c in range(N_OWN - 1, -1, -1):
                    sl = slice(1 + c * CH, 1 + (c + 1) * CH)
                    initA = (1.0 if c == N_OWN - 1
                             else b_t[i][:, 1 + (c + 1) * CH:2 + (c + 1) * CH])
                    nc.vector.tensor_tensor_scan(
                        b_t[i][:, sl][:, ::-1], a_t[i][:, sl][:, ::-1],
                        a_t[i][:, sl][:, ::-1], initA, AL.mult, AL.bypass)

            # ---- fixup: h += cumA * carry (in place over own cols) ----
            for i in range(4):
                nc.vector.scalar_tensor_tensor(
                    out=h_t[i][:, own], in0=b_t[i][:, own],
                    scalar=cu[:, i:i + 1], in1=h_t[i][:, own],
                    op0=AL.mult, op1=AL.add)
            # NOTE: b_t own cols now hold cumA; b halo cols still original b.
            for pt in range(2):
                nc.vector.tensor_copy(h_t[pt][:, 0:1], R[:, pt:pt + 1])
                nc.vector.tensor_copy(h_t[2 + pt][:, W - 1:W], R[:, 2 + pt:3 + pt])
            ext = sm.tile([P, 4], F32, tag="ext")
            for pt in range(2):
                nc.vector.tensor_copy(ext[:, pt:pt + 1], h_t[pt][:, W - 2:W - 1])
                nc.vector.tensor_copy(ext[:, 2 + pt:3 + pt], h_t[2 + pt][:, 1:2])
            for pt in range(2):
                nc.vector.scalar_tensor_tensor(
                    out=h_t[pt][:, W - 1:W], in0=a_t[pt][:, W - 1:W],
                    scalar=ext[:, pt:pt + 1], in1=b_t[pt][:, W - 1:W],
                    op0=AL.mult, op1=AL.add)
                nc.vector.scalar_tensor_tensor(
                    out=h_t[2 + pt][:, 0:1], in0=a_t[2 + pt][:, 0:1],
                    scalar=ext[:, 2 + pt:3 + pt], in1=b_t[2 + pt][:, 0:1],
                    op0=AL.mult, op1=AL.add)

            # ---- Phase A2: W_seq_out -> corr1 = alphaS * r_seq ----
            for off, w in full_chunks:
                for m in range(KD):
                    ps = psum()
                    for k in range(KD):
                        nc.tensor.matmul(ps[:, :w], wseq_t[:, k, m * P:(m + 1) * P],
                                         h_t[k][:, off:off + w],
                                         start=(k == 0), stop=(k == KD - 1))
                    nc.vector.tensor_scalar_mul(corr1[:, m, off:off + w],
                                                ps[:, :w], alphas_t[:, m:m + 1])

        # =================== Phase B ===================
        wsB = ctx.enter_context(tc.tile_pool(name="wsB", bufs=2))

        def t4B(nm):
            return wsB.tile([P, KD, CH], BF16, tag=nm, name=nm)

        # B1: pixel_norm2 + W_in -> r2 chunks [128, KD, 514]
        r2c = []
        for ci, (off, w) in enumerate(full_chunks):
            r2t = wsB.tile([P, KD, CH + 2], BF16, tag=f"r2c{ci % 3}",
                           name=f"r2c{ci}")
            r2c.append(r2t)
            x1bf = t4()
            xc = t4()
            nc.sync.dma_start(xc[:, :, :w], x_view[:, :, off:off + w])
            nc.vector.tensor_add(x1bf[:, :, :w], xc[:, :, :w],
                                 corr1[:, :, off:off + w])
            xsq = t4()
            nc.vector.tensor_mul(xsq[:, :, :w], x1bf[:, :, :w], x1bf[:, :, :w])
            ps_ss = psum()
            for k in range(KD):
                nc.tensor.matmul(ps_ss[:, :w], ones_t[:], xsq[:, k, :w],
                                 start=(k == 0), stop=(k == KD - 1))
            inorm = tf()
            nc.scalar.activation(inorm[:, :w], ps_ss[:, :w], AF.Sqrt,
                                 bias=eps_t[:], scale=1.0 / D)
            nc.vector.reciprocal_approx_fast(inorm[:, :w], inorm[:, :w])
            for m in range(KD):
                ps = psum()
                for k in range(KD):
                    nc.tensor.matmul(ps[:, :w], win_t[:, k, m * P:(m + 1) * P],
                                     x1bf[:, k, :w],
                                     start=(k == 0), stop=(k == KD - 1))
                t = tf()
                nc.vector.tensor_mul(t[:, :w], ps[:, :w], inorm[:, :w])
                nc.scalar.activation(r2t[:, m, 0:w], t[:, :w],
                                     AF.Identity, bias=bin_t[:, m:m + 1], scale=1.0)
            # halo masking + splice of this chunk's first 2 cols into chunk c-1
            if ci == 0:
                for m in range(KD):
                    nc.vector.tensor_scalar_mul(r2t[:, m, 0:1], r2t[:, m, 0:1],
                                                maskv_t[:, 0:1])
            if ci == N_FULL - 1:
                for m in range(KD):
                    nc.vector.tensor_scalar_mul(r2t[:, m, 1:2], r2t[:, m, 1:2],
                                                maskv_t[:, 1:2])
            if ci > 0:
                nc.vector.tensor_copy(r2c[ci - 1][:, :, CH:CH + 2],
                                      r2t[:, :, 0:2])

        # B2: convs, gating, W_out, residual
        for c in range(N_OWN):
            off = 1 + c * CH
            r2t = r2c[c]
            convh = t4B("convh")
            convg = t4B("convg")
            for m in range(KD):
                for dst, w3 in ((convh, dwh_t), (convg, dwg_t)):
                    nc.vector.tensor_scalar_mul(
                        dst[:, m, :], r2t[:, m, 0:CH], w3[:, m, 0:1])
                    nc.vector.scalar_tensor_tensor(
                        out=dst[:, m, :], in0=r2t[:, m, 1:CH + 1],
                        scalar=w3[:, m, 1:2], in1=dst[:, m, :],
                        op0=AL.mult, op1=AL.add)
                    nc.vector.scalar_tensor_tensor(
                        out=dst[:, m, :], in0=r2t[:, m, 2:CH + 2],
                        scalar=w3[:, m, 2:3], in1=dst[:, m, :],
                        op0=AL.mult, op1=AL.add)
            gate = t4B("gate")
            for m in range(KD):
                ps_h = psum("ppB")
                for k in range(KD):
                    nc.tensor.matmul(ps_h[:], wpwh_t[:, k, m * P:(m + 1) * P],
                                     convh[:, k, :],
                                     start=(k == 0), stop=(k == KD - 1))
                ps_g = psum("ppB")
                for k in range(KD):
                    nc.tensor.matmul(ps_g[:], wpwg_t[:, k, m * P:(m + 1) * P],
                                     convg[:, k, :],
                                     start=(k == 0), stop=(k == KD - 1))
                sg = tb()
                nc.scalar.activation(sg[:], ps_g[:], AF.Silu)
                nc.vector.tensor_mul(gate[:, m, :], ps_h[:], sg[:])
            xfc = wsB.tile([P, KD, CH], F32, tag="xf32", name="xfc")
            nc.sync.dma_start(xfc[:], xf_view[:, :, off:off + CH])
            x1c = wsB.tile([P, KD, CH], F32, tag="xf32", name="x1c")
            nc.vector.tensor_add(x1c[:], xfc[:], corr1[:, :, off:off + CH])
            for m in range(KD):
                ps_o = psum("ppB")
                for k in range(KD):
                    nc.tensor.matmul(ps_o[:], wout_t[:, k, m * P:(m + 1) * P],
                                     gate[:, k, :],
                                     start=(k == 0), stop=(k == KD - 1))
                outc = tf()
                nc.vector.scalar_tensor_tensor(
                    out=outc[:], in0=ps_o[:], scalar=alphac_t[:, m:m + 1],
                    in1=x1c[:, m, :], op0=AL.mult, op1=AL.add)
                nc.sync.dma_start(y_view[:, m, c * CH:(c + 1) * CH], outc[:])

    nc.compile()
    return nc


def _mp_w(w):
    flat = w.reshape(w.shape[0], -1).astype(np.float64)
    n = np.sqrt(np.sum(flat * flat, axis=1, keepdims=True) + 1e-8)
    return (flat / (n * np.sqrt(flat.shape[1]))).reshape(w.shape).astype(np.float32)


def _lhsT(w):
    """[O, I] weight -> [P, KD, O] lhsT tiles (I = KD*P on partitions)."""
    o, i = w.shape
    return np.ascontiguousarray(
        w.T.reshape(i // P, P, o).transpose(1, 0, 2)).astype(NPBF)


def _cols(v):
    """[M*P] vector -> [P, M] per-partition columns."""
    m = v.shape[0] // P
    return np.ascontiguousarray(v.reshape(m, P).T).astype(np.float32)


def kernel(**inputs):
    global _NC, LAST_RESULT
    if _NC is None:
        _NC = _build()
    nc = _NC

    x = np.asarray(inputs["x"], np.float32)
    c = np.asarray(inputs["c"], np.float32)
    B = x.shape[0]

    Wz_f = _mp_w(np.asarray(inputs["Wz_f"]))
    Wh_f = _mp_w(np.asarray(inputs["Wh_f"]))
    Wz_b = _mp_w(np.asarray(inputs["Wz_b"]))
    Wh_b = _mp_w(np.asarray(inputs["Wh_b"]))
    Wseq = _mp_w(np.asarray(inputs["W_seq_out"]))
    Win = _mp_w(np.asarray(inputs["W_in"]))
    Wpwh = _mp_w(np.asarray(inputs["W_pw_h"]))
    Wpwg = _mp_w(np.asarray(inputs["W_pw_g"]))
    Wout = _mp_w(np.asarray(inputs["W_out"])) / np.float32(0.596)
    dwh = _mp_w(np.asarray(inputs["W_dw_h"]))[:, 0, :]
    dwg = _mp_w(np.asarray(inputs["W_dw_g"]))[:, 0, :]
    g_seq = np.float32(np.asarray(inputs["g_seq"]))
    g_chn = np.float32(np.asarray(inputs["g_chn"]))

    ss = c @ _mp_w(np.asarray(inputs["Ws_scale"])).T
    sh = c @ _mp_w(np.asarray(inputs["Ws_shift"])).T
    sa = g_seq * (c @ _mp_w(np.asarray(inputs["Ws_alpha"])).T)
    cs = c @ _mp_w(np.asarray(inputs["Wc_scale"])).T
    chh = c @ _mp_w(np.asarray(inputs["Wc_shift"])).T
    ca = g_chn * (c @ _mp_w(np.asarray(inputs["Wc_alpha"])).T)

    A = np.concatenate([Wz_f, Wh_f, Wz_b, Wh_b], axis=0)  # [1024, 512]

    batch_maps = []
    for b in range(B):
        bm = {
            "wzh": _lhsT(A * ss[b][None, :]),
            "bzh": _cols(A @ sh[b]),
            "wseq": _lhsT(Wseq),
            "alphas": _cols(sa[b]),
            "win": _lhsT(Win * cs[b][None, :]),
            "bin": _cols(Win @ chh[b]),
            "alphac": _cols(ca[b]),
            "wpwh": _lhsT(Wpwh),
            "wpwg": _lhsT(Wpwg),
            "wout": _lhsT(Wout),
            "dwh": np.ascontiguousarray(
                dwh.reshape(KD, P, 3).transpose(1, 0, 2)).astype(np.float32),
            "dwg": np.ascontiguousarray(
                dwg.reshape(KD, P, 3).transpose(1, 0, 2)).astype(np.float32),
        }
        bm["nbzh"] = -bm["bzh"]
        batch_maps.append(bm)

    in_maps = []
    for core in range(8):
        b, s = core // 2, core % 2
        buf = np.zeros((D, W), np.float32)
        if s == 0:
            buf[:, 1:W] = x[b][:, 0:W - 1]          # global cols 0..4096
        else:
            buf[:, 0:W - 1] = x[b][:, LLOC - 1:2 * LLOC]  # global cols 4095..8191
        maskv = np.zeros((P, 2), np.float32)
        maskv[:, 0] = 1.0 if s == 1 else 0.0
        maskv[:, 1] = 1.0 if s == 0 else 0.0
        m = dict(batch_maps[b])
        m["xbf"] = buf.astype(NPBF)
        m["xf"] = buf
        m["maskv"] = maskv
        in_maps.append(m)

    res = run_bass_kernel_spmd(nc, in_maps, core_ids=list(range(8)), trace=TRACE)
    LAST_RESULT = res

    out = np.empty((B, D, 2 * LLOC), np.float32)
    for core in range(8):
        b, s = core // 2, core % 2
        out[b][:, s * LLOC:(s + 1) * LLOC] = res.results[core]["y"]
    return out
